# revision 37
# baseline (speedup 1.0000x reference)
"""Chamfer distance matrix (L2) kernel for 8 Trainium2 NeuronCores.

Problem: xyz1 [B=32, G1=64, N1=32, 3], xyz2 [B=32, G2=64, N2=32, 3] ->
out[b, g1, g2] = mean_n1 min_n2 d + mean_n2 min_n1 d, where
d[n1, n2] = |x - y|^2 between points of group (b, g1) and (b, g2).

Strategy (data-parallel over B, 4 batches per core):
  - Host packs points into augmented 5-vectors so one K=5 fp16 matmul
    produces the full pairwise squared-distance matrix:
      X' = (|x|^2, 1, -2x),  Y' = (1, |y|^2, y),  d = X'.Y'
    lhsT layout (b, g, n) and rhs layout (b, half, g, n_half) copies are
    shipped so every matmul reads a contiguous [5, 512] rhs slice.
  - K=5 uses only 5 of the PE array's 128 rows, so the 4 matmuls of a
    tile are packed into 4 distinct 32-row bands (tile_position row
    tiling) and run concurrently: operands are replicated at partition
    offsets 0/32/64/96 on chip.
  - Min over n2: ACT stages one PSUM half into SBUF (walrus rejects
    dual-PSUM TensorTensor), DVE does an elementwise min of the halves
    (fp32->fp16) per tile, then a pairwise fp16 2x min chain batched
    over two consecutive tiles, writing mins into a per-side buffer.
  - Mean over the 32 points of each group: TWO wide matmuls per side
    with the block-diagonal (1/32) matrix STATIONARY (columns are
    contracted independently), instead of 16 tiny per-tile matmuls.
    The [4, 512] mean grids go straight to DRAM; the host adds
    Z_A + Z_B^T. This keeps the PE stream free of waits on the DVE
    min chain and removes the on-chip transpose/add epilogue.
  - Both orientations (min over n2, min over n1) run the same way with
    lhsT/rhs swapped.
"""

import functools
import numpy as np

import concourse.bass as bass
import concourse.tile as tile
from concourse import bacc, mybir
from concourse import bass_utils

F32 = mybir.dt.float32
F16 = mybir.dt.float16
MIN = mybir.AluOpType.min

B, G, N = 32, 64, 32
NCORES = 8
BPC = B // NCORES          # batches per core
PTS = BPC * G * N          # points per core per set (8192)
GBLK = G // 4              # 16 blocks of 4 groups

CONFIG = {
    "row_tiling": True,   # pack the 4 d-matmuls into 4 PE row bands
    "f16_frac": 7,        # of 16 tiles: ACT double-stages both halves as
                          # fp16 so DVE's L1 min runs in 2x mode
}

# Set by test.py to collect an NTFF profile + exec time.
TRACE = False
TRACE_DIR = None
LAST_EXEC_NS = None
LAST_RESULT = None


def _cfg_key(cfg):
    return tuple(sorted(cfg.items()))


@functools.lru_cache(maxsize=4)
def _build(cfg_items):
    cfg = dict(cfg_items)
    row_tiling = cfg["row_tiling"]
    f16_frac = cfg["f16_frac"]
    nreps = 4 if row_tiling else 1
    nc = bacc.Bacc(
        "TRN2", target_bir_lowering=False, debug=False, enable_asserts=False
    )
    xal_d = nc.dram_tensor("xal", [5, PTS], F16, kind="ExternalInput")
    yal_d = nc.dram_tensor("yal", [5, PTS], F16, kind="ExternalInput")
    xar_d = nc.dram_tensor("xar", [5, PTS], F16, kind="ExternalInput")
    yar_d = nc.dram_tensor("yar", [5, PTS], F16, kind="ExternalInput")
    bo_d = nc.dram_tensor("bo", [128, 4], F16, kind="ExternalInput")
    z_d = nc.dram_tensor("z", [BPC, 2, 2, 4, 512], F32, kind="ExternalOutput")

    with tile.TileContext(nc) as tc:
        with (
            tc.tile_pool(name="const", bufs=1) as cpool,
            tc.tile_pool(name="dpsum", bufs=3, space="PSUM") as dpool,
            tc.tile_pool(name="zpsum", bufs=2, space="PSUM") as zpool,
            tc.tile_pool(name="s1", bufs=6) as s1pool,
            tc.tile_pool(name="s0f", bufs=5) as s0fpool,
            tc.tile_pool(name="s1f", bufs=5) as s1fpool,
            tc.tile_pool(name="t1", bufs=4) as t1pool,
            tc.tile_pool(name="t2", bufs=4) as t2pool,
            tc.tile_pool(name="t3", bufs=2) as t3pool,
            tc.tile_pool(name="t4", bufs=2) as t4pool,
            tc.tile_pool(name="m", bufs=2) as mpool,
            tc.tile_pool(name="zs", bufs=2) as zspool,
        ):
            BO = cpool.tile([128, 4], F16)
            nc.sync.dma_start(BO[:], bo_d.ap()[:])
            # Operands replicated at partition offsets 0/32/64/96 for the
            # 4 PE row bands; per-batch chunks so batch 0 starts early.
            npart = 32 * (nreps - 1) + 5
            XL = cpool.tile([npart, PTS], F16)
            YL = cpool.tile([npart, PTS], F16)
            XR = cpool.tile([npart, PTS], F16)
            YR = cpool.tile([npart, PTS], F16)
            for b in range(BPC):
                # Band r of a rhs replica only ever reads its own 512-col
                # quarter (cols = b*2048 + r*512), so load just that.
                for r in range(nreps):
                    po = 32 * r
                    qsl = slice(b * 2048 + r * 512, b * 2048 + (r + 1) * 512)
                    engy = nc.scalar if b == 0 else nc.gpsimd
                    engy.dma_start(YR[po : po + 5, qsl], yar_d.ap()[:, qsl])
                    nc.gpsimd.dma_start(XR[po : po + 5, qsl], xar_d.ap()[:, qsl])
                # lhsT replicas need the full batch slice on every band;
                # batch 0 arrives in half chunks so matmuls start sooner.
                chunks = (
                    [slice(0, 1024), slice(1024, 2048)] if b == 0 else
                    [slice(b * 2048, (b + 1) * 2048)]
                )
                for sl in chunks:
                    for r in range(nreps):
                        po = 32 * r
                        engx = nc.sync if b == 0 else nc.gpsimd
                        engx.dma_start(XL[po : po + 5, sl], xal_d.ap()[:, sl])
                        nc.gpsimd.dma_start(YL[po : po + 5, sl], yal_d.ap()[:, sl])

            def d_tiles(b, i, L, R):
                """PE: the 4 [5,128]x[5,512] matmuls of tile (b, i)."""
                dh = []
                for h in range(2):
                    t = dpool.tile([128, 1024], F32, tag="d")
                    base = b * 2048 + h * 1024
                    for k in range(2):
                        r = 2 * h + k if row_tiling else 0
                        po = 32 * r
                        lhsT = L[
                            po : po + 5, b * 2048 + i * 128 : b * 2048 + (i + 1) * 128
                        ]
                        rhs = R[po : po + 5, base + 512 * k : base + 512 * (k + 1)]
                        nc.tensor.matmul(
                            t[:, 512 * k : 512 * (k + 1)],
                            lhsT,
                            rhs,
                            start=True,
                            stop=True,
                            tile_position=(po, 0) if row_tiling else None,
                        )
                    dh.append(t)
                return dh

            def l1(dh, staged):
                """ACT stages PSUM halves; returns DVE L1 operands."""
                if staged:
                    sb0 = s0fpool.tile([128, 1024], F16)
                    nc.scalar.copy(sb0[:], dh[0][:])
                    sb1 = s1fpool.tile([128, 1024], F16)
                    nc.scalar.copy(sb1[:], dh[1][:])
                    return sb0, sb1
                # Copy h0 (ready half a tile earlier) and keep h1 in PSUM:
                # h0's banks free at copy time instead of at the L1 min,
                # cutting PSUM residency and letting ACT start sooner.
                sb0 = s1pool.tile([128, 1024], F32)
                nc.scalar.copy(sb0[:], dh[0][:])
                return dh[1], sb0

            def tail(t1, width, mdst):
                """Pairwise fp16 min chain over (width) groups of 16."""
                t1v = t1[:].rearrange("p (g n) -> p g n", n=16)
                t2 = t2pool.tile([128, width * 8], F16, tag="t2")
                t2v = t2[:].rearrange("p (g n) -> p g n", n=8)
                nc.vector.tensor_tensor(t2v, t1v[:, :, 0:8], t1v[:, :, 8:16], op=MIN)
                t3 = t3pool.tile([128, width * 4], F16, tag="t3")
                t3v = t3[:].rearrange("p (g n) -> p g n", n=4)
                nc.vector.tensor_tensor(t3v, t2v[:, :, 0:4], t2v[:, :, 4:8], op=MIN)
                t4 = t4pool.tile([128, width * 2], F16, tag="t4")
                t4v = t4[:].rearrange("p (g n) -> p g n", n=2)
                nc.vector.tensor_tensor(t4v, t3v[:, :, 0:2], t3v[:, :, 2:4], op=MIN)
                nc.vector.tensor_tensor(mdst, t4v[:, :, 0], t4v[:, :, 1], op=MIN)

            tile_counter = [0]

            def next_staged():
                idx = tile_counter[0]
                tile_counter[0] += 1
                return (idx * f16_frac) % 16 + f16_frac > 16

            def mean_chunk(b, side, MM, c):
                Z = zpool.tile([4, 512], F32, tag="z")
                nc.tensor.matmul(
                    Z[:], BO[:], MM[:, 512 * c : 512 * (c + 1)],
                    start=True, stop=True,
                )
                ZS = zspool.tile([4, 512], F32, tag="zs")
                nc.scalar.copy(ZS[:], Z[:])
                nc.gpsimd.dma_start(z_d.ap()[b, side, c], ZS[:])

            def run_side(b, side, L, R, last=False):
                MM = mpool.tile([128, 1024], F16, tag=f"m{side}")
                for ip in range(GBLK // 2):
                    t1 = t1pool.tile([128, 2048], F16, tag="t1")
                    for tau in range(2):
                        i = 2 * ip + tau
                        dh = d_tiles(b, i, L, R)
                        d0, sb1 = l1(dh, next_staged())
                        nc.vector.tensor_tensor(
                            t1[:, 1024 * tau : 1024 * (tau + 1)],
                            d0[:],
                            sb1[:],
                            op=MIN,
                        )
                    tail(t1, 128, MM[:, 128 * ip : 128 * (ip + 1)])
                    if last and ip == 5:
                        # Last side: its first mean half (pairs 0-3) is
                        # ready; emit it mid-side to shorten the drain tail.
                        mean_chunk(b, side, MM, 0)

                # Means: two wide matmuls, BO stationary, columns of MM
                # (tile, group) are contracted independently over the
                # 32-point partition blocks. Emitted one side late so the
                # PE stream never waits on the DVE min chain.
                def means():
                    for c in ([1] if last else [0, 1]):
                        mean_chunk(b, side, MM, c)

                return means

            pending = None
            for b in range(BPC):
                for side in range(2):
                    L, R = (XL, YR) if side == 0 else (YL, XR)
                    last = b == BPC - 1 and side == 1
                    means = run_side(b, side, L[:], R[:], last=last)
                    if pending is not None:
                        pending()
                    pending = means
            pending()

    nc.compile()
    return nc


def _host_prep(xyz1, xyz2):
    x = np.ascontiguousarray(xyz1, dtype=np.float32).reshape(B * G * N, 3)
    y = np.ascontiguousarray(xyz2, dtype=np.float32).reshape(B * G * N, 3)
    xa = np.empty((5, B * G * N), np.float16)
    xa[0] = (x * x).sum(-1)
    xa[1] = 1.0
    xa[2:5] = -2.0 * x.T
    ya = np.empty((5, B * G * N), np.float16)
    ya[0] = 1.0
    ya[1] = (y * y).sum(-1)
    ya[2:5] = y.T
    # rhs layout: (b, g, h nh) -> (b, h, g, nh)
    xar = (
        xa.reshape(5, B, G, 2, N // 2).transpose(0, 1, 3, 2, 4).reshape(5, -1)
    )
    yar = (
        ya.reshape(5, B, G, 2, N // 2).transpose(0, 1, 3, 2, 4).reshape(5, -1)
    )
    bo = np.zeros((128, 4), np.float16)
    for mblk in range(4):
        bo[32 * mblk : 32 * (mblk + 1), mblk] = 1.0 / 32
    return xa, ya, xar, yar, bo


def _assemble(z):
    """z [BPC, 2 sides, 2, 4, 512] -> out [BPC, 64, 64].

    Side 0: rows = g1 points -> z[b,0,c,sub,(i,g2)]: g1 = (c*8+i)*4+sub.
    Side 1: rows = g2 points -> z[b,1,c,sub,(j,g1)]: g2 = (c*8+j)*4+sub.
    """
    zc = z.reshape(BPC, 2, 2, 4, 8, 64)           # [b, side, c, sub, i, gcol]
    zc = zc.transpose(0, 1, 2, 4, 3, 5)           # [b, side, c, i, sub, gcol]
    zc = zc.reshape(BPC, 2, 64, 64)               # [b, side, grow, gcol]
    return zc[:, 0] + zc[:, 1].transpose(0, 2, 1)


def kernel(xyz1_matrix, xyz2_matrix):
    global LAST_EXEC_NS, LAST_RESULT
    xal, yal, xar, yar, bo = _host_prep(
        np.asarray(xyz1_matrix), np.asarray(xyz2_matrix)
    )
    nc = _build(_cfg_key(CONFIG))
    in_maps = []
    for c in range(NCORES):
        sl = slice(c * PTS, (c + 1) * PTS)
        in_maps.append(
            {
                "xal": np.ascontiguousarray(xal[:, sl]),
                "yal": np.ascontiguousarray(yal[:, sl]),
                "xar": np.ascontiguousarray(xar[:, sl]),
                "yar": np.ascontiguousarray(yar[:, sl]),
                "bo": bo,
            }
        )
    res = bass_utils.run_bass_kernel_spmd(
        nc, in_maps, core_ids=list(range(NCORES)), trace=TRACE, tmpdir=TRACE_DIR
    )
    LAST_RESULT = res
    LAST_EXEC_NS = res.exec_time_ns
    outs = [_assemble(r["z"]) for r in res.results]
    return np.concatenate(outs, axis=0).astype(np.float32)


# revision 38
# speedup vs baseline: 1.0117x; 1.0117x over previous
"""Chamfer distance matrix (L2) kernel for 8 Trainium2 NeuronCores.

Problem: xyz1 [B=32, G1=64, N1=32, 3], xyz2 [B=32, G2=64, N2=32, 3] ->
out[b, g1, g2] = mean_n1 min_n2 d + mean_n2 min_n1 d, where
d[n1, n2] = |x - y|^2 between points of group (b, g1) and (b, g2).

Strategy (data-parallel over B, 4 batches per core):
  - Host packs points into augmented 5-vectors so one K=5 fp16 matmul
    produces the full pairwise squared-distance matrix:
      X' = (|x|^2, 1, -2x),  Y' = (1, |y|^2, y),  d = X'.Y'
    lhsT layout (b, g, n) and rhs layout (b, half, g, n_half) copies are
    shipped so every matmul reads a contiguous [5, 512] rhs slice.
  - K=5 uses only 5 of the PE array's 128 rows, so the 4 matmuls of a
    tile are packed into 4 distinct 32-row bands (tile_position row
    tiling) and run concurrently: operands are replicated at partition
    offsets 0/32/64/96 on chip.
  - Min over n2: ACT stages one PSUM half into SBUF (walrus rejects
    dual-PSUM TensorTensor), DVE does an elementwise min of the halves
    (fp32->fp16) per tile, then a pairwise fp16 2x min chain batched
    over two consecutive tiles, writing mins into a per-side buffer.
  - Mean over the 32 points of each group: TWO wide matmuls per side
    with the block-diagonal (1/32) matrix STATIONARY (columns are
    contracted independently), instead of 16 tiny per-tile matmuls.
    The [4, 512] mean grids go straight to DRAM; the host adds
    Z_A + Z_B^T. This keeps the PE stream free of waits on the DVE
    min chain and removes the on-chip transpose/add epilogue.
  - Both orientations (min over n2, min over n1) run the same way with
    lhsT/rhs swapped.
"""

import functools
import numpy as np

import concourse.bass as bass
import concourse.tile as tile
from concourse import bacc, mybir
from concourse import bass_utils

F32 = mybir.dt.float32
F16 = mybir.dt.float16
MIN = mybir.AluOpType.min

B, G, N = 32, 64, 32
NCORES = 8
BPC = B // NCORES          # batches per core
PTS = BPC * G * N          # points per core per set (8192)
GBLK = G // 4              # 16 blocks of 4 groups

CONFIG = {
    "row_tiling": True,   # pack the 4 d-matmuls into 4 PE row bands
    "f16_frac": 7,        # of 16 tiles: ACT double-stages both halves as
                          # fp16 so DVE's L1 min runs in 2x mode
}

# Set by test.py to collect an NTFF profile + exec time.
TRACE = False
TRACE_DIR = None
LAST_EXEC_NS = None
LAST_RESULT = None


def _cfg_key(cfg):
    return tuple(sorted(cfg.items()))


@functools.lru_cache(maxsize=4)
def _build(cfg_items):
    cfg = dict(cfg_items)
    row_tiling = cfg["row_tiling"]
    f16_frac = cfg["f16_frac"]
    nreps = 4 if row_tiling else 1
    nc = bacc.Bacc(
        "TRN2", target_bir_lowering=False, debug=False, enable_asserts=False
    )
    xal_d = nc.dram_tensor("xal", [5, PTS], F16, kind="ExternalInput")
    yal_d = nc.dram_tensor("yal", [5, PTS], F16, kind="ExternalInput")
    xar_d = nc.dram_tensor("xar", [5, PTS], F16, kind="ExternalInput")
    yar_d = nc.dram_tensor("yar", [5, PTS], F16, kind="ExternalInput")
    bo_d = nc.dram_tensor("bo", [128, 4], F16, kind="ExternalInput")
    z_d = nc.dram_tensor("z", [BPC, 2, 2, 4, 512], F32, kind="ExternalOutput")

    with tile.TileContext(nc) as tc:
        with (
            tc.tile_pool(name="const", bufs=1) as cpool,
            tc.tile_pool(name="dpsum", bufs=3, space="PSUM") as dpool,
            tc.tile_pool(name="zpsum", bufs=2, space="PSUM") as zpool,
            tc.tile_pool(name="s1", bufs=6) as s1pool,
            tc.tile_pool(name="s0f", bufs=5) as s0fpool,
            tc.tile_pool(name="s1f", bufs=5) as s1fpool,
            tc.tile_pool(name="t1", bufs=4) as t1pool,
            tc.tile_pool(name="t2", bufs=4) as t2pool,
            tc.tile_pool(name="t3", bufs=2) as t3pool,
            tc.tile_pool(name="t4", bufs=2) as t4pool,
            tc.tile_pool(name="m", bufs=2) as mpool,
            tc.tile_pool(name="zs", bufs=2) as zspool,
        ):
            BO = cpool.tile([128, 4], F16)
            nc.sync.dma_start(BO[:], bo_d.ap()[:])
            # Operands replicated at partition offsets 0/32/64/96 for the
            # 4 PE row bands; per-batch chunks so batch 0 starts early.
            npart = 32 * (nreps - 1) + 5
            XL = cpool.tile([npart, PTS], F16)
            YL = cpool.tile([npart, PTS], F16)
            XR = cpool.tile([npart, PTS], F16)
            YR = cpool.tile([npart, PTS], F16)
            for b in range(BPC):
                # Band r of a rhs replica only ever reads its own 512-col
                # quarter (cols = b*2048 + r*512), so load just that.
                for r in range(nreps):
                    po = 32 * r
                    qsl = slice(b * 2048 + r * 512, b * 2048 + (r + 1) * 512)
                    engy = nc.scalar if b == 0 else nc.gpsimd
                    engy.dma_start(YR[po : po + 5, qsl], yar_d.ap()[:, qsl])
                    nc.gpsimd.dma_start(XR[po : po + 5, qsl], xar_d.ap()[:, qsl])
                # lhsT replicas need the full batch slice on every band;
                # batch 0 arrives in half chunks so matmuls start sooner.
                chunks = (
                    [slice(0, 1024), slice(1024, 2048)] if b == 0 else
                    [slice(b * 2048, (b + 1) * 2048)]
                )
                for sl in chunks:
                    for r in range(nreps):
                        po = 32 * r
                        engx = nc.sync if b == 0 else nc.gpsimd
                        engx.dma_start(XL[po : po + 5, sl], xal_d.ap()[:, sl])
                        nc.gpsimd.dma_start(YL[po : po + 5, sl], yal_d.ap()[:, sl])

            def d_tiles(b, i, L, R):
                """PE: the 4 [5,128]x[5,512] matmuls of tile (b, i)."""
                dh = []
                for h in range(2):
                    t = dpool.tile([128, 1024], F32, tag="d")
                    base = b * 2048 + h * 1024
                    for k in range(2):
                        r = 2 * h + k if row_tiling else 0
                        po = 32 * r
                        lhsT = L[
                            po : po + 5, b * 2048 + i * 128 : b * 2048 + (i + 1) * 128
                        ]
                        rhs = R[po : po + 5, base + 512 * k : base + 512 * (k + 1)]
                        nc.tensor.matmul(
                            t[:, 512 * k : 512 * (k + 1)],
                            lhsT,
                            rhs,
                            start=True,
                            stop=True,
                            tile_position=(po, 0) if row_tiling else None,
                        )
                    dh.append(t)
                return dh

            def l1(dh, staged):
                """ACT stages PSUM halves; returns DVE L1 operands."""
                if staged:
                    sb0 = s0fpool.tile([128, 1024], F16)
                    nc.scalar.copy(sb0[:], dh[0][:])
                    sb1 = s1fpool.tile([128, 1024], F16)
                    nc.scalar.copy(sb1[:], dh[1][:])
                    return sb0, sb1
                sb1 = s1pool.tile([128, 1024], F32)
                nc.scalar.copy(sb1[:], dh[1][:])
                return dh[0], sb1

            def tail(t1, width, mdst):
                """Pairwise fp16 min chain over (width) groups of 16."""
                t1v = t1[:].rearrange("p (g n) -> p g n", n=16)
                t2 = t2pool.tile([128, width * 8], F16, tag="t2")
                t2v = t2[:].rearrange("p (g n) -> p g n", n=8)
                nc.vector.tensor_tensor(t2v, t1v[:, :, 0:8], t1v[:, :, 8:16], op=MIN)
                t3 = t3pool.tile([128, width * 4], F16, tag="t3")
                t3v = t3[:].rearrange("p (g n) -> p g n", n=4)
                nc.vector.tensor_tensor(t3v, t2v[:, :, 0:4], t2v[:, :, 4:8], op=MIN)
                t4 = t4pool.tile([128, width * 2], F16, tag="t4")
                t4v = t4[:].rearrange("p (g n) -> p g n", n=2)
                nc.vector.tensor_tensor(t4v, t3v[:, :, 0:2], t3v[:, :, 2:4], op=MIN)
                nc.vector.tensor_tensor(mdst, t4v[:, :, 0], t4v[:, :, 1], op=MIN)

            tile_counter = [0]

            def next_staged():
                idx = tile_counter[0]
                tile_counter[0] += 1
                return (idx * f16_frac) % 16 + f16_frac > 16

            def mean_chunk(b, side, MM, c):
                Z = zpool.tile([4, 512], F32, tag="z")
                nc.tensor.matmul(
                    Z[:], BO[:], MM[:, 512 * c : 512 * (c + 1)],
                    start=True, stop=True,
                )
                ZS = zspool.tile([4, 512], F32, tag="zs")
                nc.scalar.copy(ZS[:], Z[:])
                nc.gpsimd.dma_start(z_d.ap()[b, side, c], ZS[:])

            def run_side(b, side, L, R, last=False):
                MM = mpool.tile([128, 1024], F16, tag=f"m{side}")
                for ip in range(GBLK // 2):
                    t1 = t1pool.tile([128, 2048], F16, tag="t1")
                    for tau in range(2):
                        i = 2 * ip + tau
                        dh = d_tiles(b, i, L, R)
                        d0, sb1 = l1(dh, next_staged())
                        nc.vector.tensor_tensor(
                            t1[:, 1024 * tau : 1024 * (tau + 1)],
                            d0[:],
                            sb1[:],
                            op=MIN,
                        )
                    tail(t1, 128, MM[:, 128 * ip : 128 * (ip + 1)])
                    if last and ip == 5:
                        # Last side: its first mean half (pairs 0-3) is
                        # ready; emit it mid-side to shorten the drain tail.
                        mean_chunk(b, side, MM, 0)

                # Means: two wide matmuls, BO stationary, columns of MM
                # (tile, group) are contracted independently over the
                # 32-point partition blocks. Emitted one side late so the
                # PE stream never waits on the DVE min chain.
                def means():
                    for c in ([1] if last else [0, 1]):
                        mean_chunk(b, side, MM, c)

                return means

            pending = None
            for b in range(BPC):
                for side in range(2):
                    L, R = (XL, YR) if side == 0 else (YL, XR)
                    last = b == BPC - 1 and side == 1
                    means = run_side(b, side, L[:], R[:], last=last)
                    if pending is not None:
                        pending()
                    pending = means
            pending()

    nc.compile()
    return nc


def _host_prep(xyz1, xyz2):
    x = np.ascontiguousarray(xyz1, dtype=np.float32).reshape(B * G * N, 3)
    y = np.ascontiguousarray(xyz2, dtype=np.float32).reshape(B * G * N, 3)
    xa = np.empty((5, B * G * N), np.float16)
    xa[0] = (x * x).sum(-1)
    xa[1] = 1.0
    xa[2:5] = -2.0 * x.T
    ya = np.empty((5, B * G * N), np.float16)
    ya[0] = 1.0
    ya[1] = (y * y).sum(-1)
    ya[2:5] = y.T
    # rhs layout: (b, g, h nh) -> (b, h, g, nh)
    xar = (
        xa.reshape(5, B, G, 2, N // 2).transpose(0, 1, 3, 2, 4).reshape(5, -1)
    )
    yar = (
        ya.reshape(5, B, G, 2, N // 2).transpose(0, 1, 3, 2, 4).reshape(5, -1)
    )
    bo = np.zeros((128, 4), np.float16)
    for mblk in range(4):
        bo[32 * mblk : 32 * (mblk + 1), mblk] = 1.0 / 32
    return xa, ya, xar, yar, bo


def _assemble(z):
    """z [BPC, 2 sides, 2, 4, 512] -> out [BPC, 64, 64].

    Side 0: rows = g1 points -> z[b,0,c,sub,(i,g2)]: g1 = (c*8+i)*4+sub.
    Side 1: rows = g2 points -> z[b,1,c,sub,(j,g1)]: g2 = (c*8+j)*4+sub.
    """
    zc = z.reshape(BPC, 2, 2, 4, 8, 64)           # [b, side, c, sub, i, gcol]
    zc = zc.transpose(0, 1, 2, 4, 3, 5)           # [b, side, c, i, sub, gcol]
    zc = zc.reshape(BPC, 2, 64, 64)               # [b, side, grow, gcol]
    return zc[:, 0] + zc[:, 1].transpose(0, 2, 1)


def kernel(xyz1_matrix, xyz2_matrix):
    global LAST_EXEC_NS, LAST_RESULT
    xal, yal, xar, yar, bo = _host_prep(
        np.asarray(xyz1_matrix), np.asarray(xyz2_matrix)
    )
    nc = _build(_cfg_key(CONFIG))
    in_maps = []
    for c in range(NCORES):
        sl = slice(c * PTS, (c + 1) * PTS)
        in_maps.append(
            {
                "xal": np.ascontiguousarray(xal[:, sl]),
                "yal": np.ascontiguousarray(yal[:, sl]),
                "xar": np.ascontiguousarray(xar[:, sl]),
                "yar": np.ascontiguousarray(yar[:, sl]),
                "bo": bo,
            }
        )
    res = bass_utils.run_bass_kernel_spmd(
        nc, in_maps, core_ids=list(range(NCORES)), trace=TRACE, tmpdir=TRACE_DIR
    )
    LAST_RESULT = res
    LAST_EXEC_NS = res.exec_time_ns
    outs = [_assemble(r["z"]) for r in res.results]
    return np.concatenate(outs, axis=0).astype(np.float32)


# revision 39
# speedup vs baseline: 1.0344x; 1.0224x over previous
"""Chamfer distance matrix (L2) kernel for 8 Trainium2 NeuronCores.

Problem: xyz1 [B=32, G1=64, N1=32, 3], xyz2 [B=32, G2=64, N2=32, 3] ->
out[b, g1, g2] = mean_n1 min_n2 d + mean_n2 min_n1 d, where
d[n1, n2] = |x - y|^2 between points of group (b, g1) and (b, g2).

Strategy (data-parallel over B, 4 batches per core):
  - Host packs points into augmented 5-vectors so one K=5 fp16 matmul
    produces the full pairwise squared-distance matrix:
      X' = (|x|^2, 1, -2x),  Y' = (1, |y|^2, y),  d = X'.Y'
    lhsT layout (b, g, n) and rhs layout (b, half, g, n_half) copies are
    shipped so every matmul reads a contiguous [5, 512] rhs slice.
  - K=5 uses only 5 of the PE array's 128 rows, so the 4 matmuls of a
    tile are packed into 4 distinct 32-row bands (tile_position row
    tiling) and run concurrently: operands are replicated at partition
    offsets 0/32/64/96 on chip.
  - Min over n2: ACT stages one PSUM half into SBUF (walrus rejects
    dual-PSUM TensorTensor), DVE does an elementwise min of the halves
    (fp32->fp16) per tile, then a pairwise fp16 2x min chain batched
    over two consecutive tiles, writing mins into a per-side buffer.
  - Mean over the 32 points of each group: TWO wide matmuls per side
    with the block-diagonal (1/32) matrix STATIONARY (columns are
    contracted independently), instead of 16 tiny per-tile matmuls.
    The [4, 512] mean grids go straight to DRAM; the host adds
    Z_A + Z_B^T. This keeps the PE stream free of waits on the DVE
    min chain and removes the on-chip transpose/add epilogue.
  - Both orientations (min over n2, min over n1) run the same way with
    lhsT/rhs swapped.
"""

import functools
import numpy as np

import concourse.bass as bass
import concourse.tile as tile
from concourse import bacc, mybir
from concourse import bass_utils

F32 = mybir.dt.float32
F16 = mybir.dt.float16
MIN = mybir.AluOpType.min

B, G, N = 32, 64, 32
NCORES = 8
BPC = B // NCORES          # batches per core
PTS = BPC * G * N          # points per core per set (8192)
GBLK = G // 4              # 16 blocks of 4 groups

CONFIG = {
    "row_tiling": True,   # pack the 4 d-matmuls into 4 PE row bands
    "f16_frac": 7,        # of 16 tiles: ACT double-stages both halves as
                          # fp16 so DVE's L1 min runs in 2x mode
}

# Set by test.py to collect an NTFF profile + exec time.
TRACE = False
TRACE_DIR = None
LAST_EXEC_NS = None
LAST_RESULT = None


def _cfg_key(cfg):
    return tuple(sorted(cfg.items()))


@functools.lru_cache(maxsize=4)
def _build(cfg_items):
    cfg = dict(cfg_items)
    row_tiling = cfg["row_tiling"]
    f16_frac = cfg["f16_frac"]
    nreps = 4 if row_tiling else 1
    nc = bacc.Bacc(
        "TRN2", target_bir_lowering=False, debug=False, enable_asserts=False
    )
    xal_d = nc.dram_tensor("xal", [5, PTS], F16, kind="ExternalInput")
    yal_d = nc.dram_tensor("yal", [5, PTS], F16, kind="ExternalInput")
    xar_d = nc.dram_tensor("xar", [5, PTS], F16, kind="ExternalInput")
    yar_d = nc.dram_tensor("yar", [5, PTS], F16, kind="ExternalInput")
    bo_d = nc.dram_tensor("bo", [128, 4], F16, kind="ExternalInput")
    z_d = nc.dram_tensor("z", [BPC, 2, 2, 4, 512], F32, kind="ExternalOutput")

    with tile.TileContext(nc) as tc:
        with (
            tc.tile_pool(name="const", bufs=1) as cpool,
            tc.tile_pool(name="dpsum", bufs=3, space="PSUM") as dpool,
            tc.tile_pool(name="zpsum", bufs=2, space="PSUM") as zpool,
            tc.tile_pool(name="s1", bufs=6) as s1pool,
            tc.tile_pool(name="s0f", bufs=5) as s0fpool,
            tc.tile_pool(name="s1f", bufs=5) as s1fpool,
            tc.tile_pool(name="t1", bufs=4) as t1pool,
            tc.tile_pool(name="t2", bufs=4) as t2pool,
            tc.tile_pool(name="t3", bufs=2) as t3pool,
            tc.tile_pool(name="t4", bufs=2) as t4pool,
            tc.tile_pool(name="m", bufs=2) as mpool,
            tc.tile_pool(name="zs", bufs=2) as zspool,
        ):
            BO = cpool.tile([128, 4], F16)
            nc.sync.dma_start(BO[:], bo_d.ap()[:])
            # Operands replicated at partition offsets 0/32/64/96 for the
            # 4 PE row bands; per-batch chunks so batch 0 starts early.
            npart = 32 * (nreps - 1) + 5
            XL = cpool.tile([npart, PTS], F16)
            YL = cpool.tile([npart, PTS], F16)
            XR = cpool.tile([npart, PTS], F16)
            YR = cpool.tile([npart, PTS], F16)
            for b in range(BPC):
                # Band r of a rhs replica only ever reads its own 512-col
                # quarter (cols = b*2048 + r*512), so load just that.
                for r in range(nreps):
                    po = 32 * r
                    qsl = slice(b * 2048 + r * 512, b * 2048 + (r + 1) * 512)
                    engy = nc.scalar if b == 0 else nc.gpsimd
                    engy.dma_start(YR[po : po + 5, qsl], yar_d.ap()[:, qsl])
                    nc.gpsimd.dma_start(XR[po : po + 5, qsl], xar_d.ap()[:, qsl])
                # lhsT replicas need the full batch slice on every band;
                # batch 0 arrives in half chunks so matmuls start sooner.
                chunks = (
                    [slice(0, 1024), slice(1024, 2048)] if b == 0 else
                    [slice(b * 2048, (b + 1) * 2048)]
                )
                for sl in chunks:
                    for r in range(nreps):
                        po = 32 * r
                        engx = nc.sync if b == 0 else nc.gpsimd
                        engx.dma_start(XL[po : po + 5, sl], xal_d.ap()[:, sl])
                        nc.gpsimd.dma_start(YL[po : po + 5, sl], yal_d.ap()[:, sl])

            def d_tiles(b, i, L, R):
                """PE: the 4 [5,128]x[5,512] matmuls of tile (b, i)."""
                dh = []
                for h in range(2):
                    t = dpool.tile([128, 1024], F32, tag="d")
                    base = b * 2048 + h * 1024
                    for k in range(2):
                        r = 2 * h + k if row_tiling else 0
                        po = 32 * r
                        lhsT = L[
                            po : po + 5, b * 2048 + i * 128 : b * 2048 + (i + 1) * 128
                        ]
                        rhs = R[po : po + 5, base + 512 * k : base + 512 * (k + 1)]
                        nc.tensor.matmul(
                            t[:, 512 * k : 512 * (k + 1)],
                            lhsT,
                            rhs,
                            start=True,
                            stop=True,
                            tile_position=(po, 0) if row_tiling else None,
                        )
                    dh.append(t)
                return dh

            def l1(dh, staged):
                """ACT stages PSUM halves; returns DVE L1 operands."""
                if staged:
                    sb0 = s0fpool.tile([128, 1024], F16)
                    nc.scalar.copy(sb0[:], dh[0][:])
                    sb1 = s1fpool.tile([128, 1024], F16)
                    nc.scalar.copy(sb1[:], dh[1][:])
                    return sb0, sb1
                sb1 = s1pool.tile([128, 1024], F32)
                nc.scalar.copy(sb1[:], dh[1][:])
                return dh[0], sb1

            def tail(t1, width, mdst):
                """Pairwise fp16 min chain over (width) groups of 16."""
                t1v = t1[:].rearrange("p (g n) -> p g n", n=16)
                t2 = t2pool.tile([128, width * 8], F16, tag="t2")
                t2v = t2[:].rearrange("p (g n) -> p g n", n=8)
                nc.vector.tensor_tensor(t2v, t1v[:, :, 0:8], t1v[:, :, 8:16], op=MIN)
                t3 = t3pool.tile([128, width * 4], F16, tag="t3")
                t3v = t3[:].rearrange("p (g n) -> p g n", n=4)
                nc.vector.tensor_tensor(t3v, t2v[:, :, 0:4], t2v[:, :, 4:8], op=MIN)
                t4 = t4pool.tile([128, width * 2], F16, tag="t4")
                t4v = t4[:].rearrange("p (g n) -> p g n", n=2)
                nc.vector.tensor_tensor(t4v, t3v[:, :, 0:2], t3v[:, :, 2:4], op=MIN)
                nc.vector.tensor_tensor(mdst, t4v[:, :, 0], t4v[:, :, 1], op=MIN)

            tile_counter = [0]

            def next_staged():
                idx = tile_counter[0]
                tile_counter[0] += 1
                return (idx * f16_frac) % 16 + f16_frac > 16

            def mean_chunk(b, side, MM, c):
                Z = zpool.tile([4, 512], F32, tag="z")
                nc.tensor.matmul(
                    Z[:], BO[:], MM[:, 512 * c : 512 * (c + 1)],
                    start=True, stop=True,
                )
                ZS = zspool.tile([4, 512], F32, tag="zs")
                nc.scalar.copy(ZS[:], Z[:])
                # SP queue: idle at the drain tail, unlike gpsimd which
                # also carries the input loads.
                nc.sync.dma_start(z_d.ap()[b, side, c], ZS[:])

            def run_side(b, side, L, R, last=False):
                MM = mpool.tile([128, 1024], F16, tag=f"m{side}")
                for ip in range(GBLK // 2):
                    t1 = t1pool.tile([128, 2048], F16, tag="t1")
                    for tau in range(2):
                        i = 2 * ip + tau
                        dh = d_tiles(b, i, L, R)
                        d0, sb1 = l1(dh, next_staged())
                        nc.vector.tensor_tensor(
                            t1[:, 1024 * tau : 1024 * (tau + 1)],
                            d0[:],
                            sb1[:],
                            op=MIN,
                        )
                    tail(t1, 128, MM[:, 128 * ip : 128 * (ip + 1)])
                    if last and ip == 5:
                        # Last side: its first mean half (pairs 0-3) is
                        # ready; emit it mid-side to shorten the drain tail.
                        mean_chunk(b, side, MM, 0)

                # Means: two wide matmuls, BO stationary, columns of MM
                # (tile, group) are contracted independently over the
                # 32-point partition blocks. Emitted one side late so the
                # PE stream never waits on the DVE min chain.
                def means():
                    for c in ([1] if last else [0, 1]):
                        mean_chunk(b, side, MM, c)

                return means

            pending = None
            for b in range(BPC):
                for side in range(2):
                    L, R = (XL, YR) if side == 0 else (YL, XR)
                    last = b == BPC - 1 and side == 1
                    means = run_side(b, side, L[:], R[:], last=last)
                    if pending is not None:
                        pending()
                    pending = means
            pending()

    nc.compile()
    return nc


def _host_prep(xyz1, xyz2):
    x = np.ascontiguousarray(xyz1, dtype=np.float32).reshape(B * G * N, 3)
    y = np.ascontiguousarray(xyz2, dtype=np.float32).reshape(B * G * N, 3)
    xa = np.empty((5, B * G * N), np.float16)
    xa[0] = (x * x).sum(-1)
    xa[1] = 1.0
    xa[2:5] = -2.0 * x.T
    ya = np.empty((5, B * G * N), np.float16)
    ya[0] = 1.0
    ya[1] = (y * y).sum(-1)
    ya[2:5] = y.T
    # rhs layout: (b, g, h nh) -> (b, h, g, nh)
    xar = (
        xa.reshape(5, B, G, 2, N // 2).transpose(0, 1, 3, 2, 4).reshape(5, -1)
    )
    yar = (
        ya.reshape(5, B, G, 2, N // 2).transpose(0, 1, 3, 2, 4).reshape(5, -1)
    )
    bo = np.zeros((128, 4), np.float16)
    for mblk in range(4):
        bo[32 * mblk : 32 * (mblk + 1), mblk] = 1.0 / 32
    return xa, ya, xar, yar, bo


def _assemble(z):
    """z [BPC, 2 sides, 2, 4, 512] -> out [BPC, 64, 64].

    Side 0: rows = g1 points -> z[b,0,c,sub,(i,g2)]: g1 = (c*8+i)*4+sub.
    Side 1: rows = g2 points -> z[b,1,c,sub,(j,g1)]: g2 = (c*8+j)*4+sub.
    """
    zc = z.reshape(BPC, 2, 2, 4, 8, 64)           # [b, side, c, sub, i, gcol]
    zc = zc.transpose(0, 1, 2, 4, 3, 5)           # [b, side, c, i, sub, gcol]
    zc = zc.reshape(BPC, 2, 64, 64)               # [b, side, grow, gcol]
    return zc[:, 0] + zc[:, 1].transpose(0, 2, 1)


def kernel(xyz1_matrix, xyz2_matrix):
    global LAST_EXEC_NS, LAST_RESULT
    xal, yal, xar, yar, bo = _host_prep(
        np.asarray(xyz1_matrix), np.asarray(xyz2_matrix)
    )
    nc = _build(_cfg_key(CONFIG))
    in_maps = []
    for c in range(NCORES):
        sl = slice(c * PTS, (c + 1) * PTS)
        in_maps.append(
            {
                "xal": np.ascontiguousarray(xal[:, sl]),
                "yal": np.ascontiguousarray(yal[:, sl]),
                "xar": np.ascontiguousarray(xar[:, sl]),
                "yar": np.ascontiguousarray(yar[:, sl]),
                "bo": bo,
            }
        )
    res = bass_utils.run_bass_kernel_spmd(
        nc, in_maps, core_ids=list(range(NCORES)), trace=TRACE, tmpdir=TRACE_DIR
    )
    LAST_RESULT = res
    LAST_EXEC_NS = res.exec_time_ns
    outs = [_assemble(r["z"]) for r in res.results]
    return np.concatenate(outs, axis=0).astype(np.float32)


# revision 40
# speedup vs baseline: 1.0370x; 1.0025x over previous
"""Chamfer distance matrix (L2) kernel for 8 Trainium2 NeuronCores.

Problem: xyz1 [B=32, G1=64, N1=32, 3], xyz2 [B=32, G2=64, N2=32, 3] ->
out[b, g1, g2] = mean_n1 min_n2 d + mean_n2 min_n1 d, where
d[n1, n2] = |x - y|^2 between points of group (b, g1) and (b, g2).

Strategy (data-parallel over B, 4 batches per core):
  - Host packs points into augmented 5-vectors so one K=5 fp16 matmul
    produces the full pairwise squared-distance matrix:
      X' = (|x|^2, 1, -2x),  Y' = (1, |y|^2, y),  d = X'.Y'
    lhsT layout (b, g, n) and rhs layout (b, half, g, n_half) copies are
    shipped so every matmul reads a contiguous [5, 512] rhs slice.
  - K=5 uses only 5 of the PE array's 128 rows, so the 4 matmuls of a
    tile are packed into 4 distinct 32-row bands (tile_position row
    tiling) and run concurrently: operands are replicated at partition
    offsets 0/32/64/96 on chip.
  - Min over n2: ACT stages one PSUM half into SBUF (walrus rejects
    dual-PSUM TensorTensor), DVE does an elementwise min of the halves
    (fp32->fp16) per tile, then a pairwise fp16 2x min chain batched
    over two consecutive tiles, writing mins into a per-side buffer.
  - Mean over the 32 points of each group: TWO wide matmuls per side
    with the block-diagonal (1/32) matrix STATIONARY (columns are
    contracted independently), instead of 16 tiny per-tile matmuls.
    The [4, 512] mean grids go straight to DRAM; the host adds
    Z_A + Z_B^T. This keeps the PE stream free of waits on the DVE
    min chain and removes the on-chip transpose/add epilogue.
  - Both orientations (min over n2, min over n1) run the same way with
    lhsT/rhs swapped.
"""

import functools
import numpy as np

import concourse.bass as bass
import concourse.tile as tile
from concourse import bacc, mybir
from concourse import bass_utils

F32 = mybir.dt.float32
F16 = mybir.dt.float16
MIN = mybir.AluOpType.min

B, G, N = 32, 64, 32
NCORES = 8
BPC = B // NCORES          # batches per core
PTS = BPC * G * N          # points per core per set (8192)
GBLK = G // 4              # 16 blocks of 4 groups

CONFIG = {
    "row_tiling": True,   # pack the 4 d-matmuls into 4 PE row bands
    "f16_frac": 7,        # of 16 tiles: ACT double-stages both halves as
                          # fp16 so DVE's L1 min runs in 2x mode
}

# Set by test.py to collect an NTFF profile + exec time.
TRACE = False
TRACE_DIR = None
LAST_EXEC_NS = None
LAST_RESULT = None


def _cfg_key(cfg):
    return tuple(sorted(cfg.items()))


@functools.lru_cache(maxsize=4)
def _build(cfg_items):
    cfg = dict(cfg_items)
    row_tiling = cfg["row_tiling"]
    f16_frac = cfg["f16_frac"]
    nreps = 4 if row_tiling else 1
    nc = bacc.Bacc(
        "TRN2", target_bir_lowering=False, debug=False, enable_asserts=False
    )
    xal_d = nc.dram_tensor("xal", [5, PTS], F16, kind="ExternalInput")
    yal_d = nc.dram_tensor("yal", [5, PTS], F16, kind="ExternalInput")
    xar_d = nc.dram_tensor("xar", [5, PTS], F16, kind="ExternalInput")
    yar_d = nc.dram_tensor("yar", [5, PTS], F16, kind="ExternalInput")
    bo_d = nc.dram_tensor("bo", [128, 4], F16, kind="ExternalInput")
    z_d = nc.dram_tensor("z", [BPC, 2, 2, 4, 512], F32, kind="ExternalOutput")

    with tile.TileContext(nc) as tc:
        with (
            tc.tile_pool(name="const", bufs=1) as cpool,
            tc.tile_pool(name="dpsum", bufs=3, space="PSUM") as dpool,
            tc.tile_pool(name="zpsum", bufs=2, space="PSUM") as zpool,
            tc.tile_pool(name="s1", bufs=6) as s1pool,
            tc.tile_pool(name="s0f", bufs=5) as s0fpool,
            tc.tile_pool(name="s1f", bufs=5) as s1fpool,
            tc.tile_pool(name="t1", bufs=4) as t1pool,
            tc.tile_pool(name="t2", bufs=4) as t2pool,
            tc.tile_pool(name="t3", bufs=2) as t3pool,
            tc.tile_pool(name="t4", bufs=2) as t4pool,
            tc.tile_pool(name="m", bufs=2) as mpool,
            tc.tile_pool(name="zs", bufs=2) as zspool,
        ):
            BO = cpool.tile([128, 4], F16)
            nc.sync.dma_start(BO[:], bo_d.ap()[:])
            # Operands replicated at partition offsets 0/32/64/96 for the
            # 4 PE row bands; per-batch chunks so batch 0 starts early.
            npart = 32 * (nreps - 1) + 5
            XL = cpool.tile([npart, PTS], F16)
            YL = cpool.tile([npart, PTS], F16)
            XR = cpool.tile([npart, PTS], F16)
            YR = cpool.tile([npart, PTS], F16)
            for b in range(BPC):
                # Band r of a rhs replica only ever reads its own 512-col
                # quarter (cols = b*2048 + r*512), so load just that.
                for r in range(nreps):
                    po = 32 * r
                    qsl = slice(b * 2048 + r * 512, b * 2048 + (r + 1) * 512)
                    engy = nc.scalar if b == 0 else nc.gpsimd
                    engy.dma_start(YR[po : po + 5, qsl], yar_d.ap()[:, qsl])
                    nc.gpsimd.dma_start(XR[po : po + 5, qsl], xar_d.ap()[:, qsl])
                # lhsT replicas need the full batch slice on every band;
                # batch 0 arrives in half chunks so matmuls start sooner.
                chunks = (
                    [slice(0, 1024), slice(1024, 2048)] if b == 0 else
                    [slice(b * 2048, (b + 1) * 2048)]
                )
                for sl in chunks:
                    for r in range(nreps):
                        po = 32 * r
                        engx = nc.sync if b == 0 else nc.gpsimd
                        engx.dma_start(XL[po : po + 5, sl], xal_d.ap()[:, sl])
                        nc.gpsimd.dma_start(YL[po : po + 5, sl], yal_d.ap()[:, sl])

            def d_tiles(b, i, L, R):
                """PE: the 4 [5,128]x[5,512] matmuls of tile (b, i)."""
                dh = []
                for h in range(2):
                    t = dpool.tile([128, 1024], F32, tag="d")
                    base = b * 2048 + h * 1024
                    for k in range(2):
                        r = 2 * h + k if row_tiling else 0
                        po = 32 * r
                        lhsT = L[
                            po : po + 5, b * 2048 + i * 128 : b * 2048 + (i + 1) * 128
                        ]
                        rhs = R[po : po + 5, base + 512 * k : base + 512 * (k + 1)]
                        nc.tensor.matmul(
                            t[:, 512 * k : 512 * (k + 1)],
                            lhsT,
                            rhs,
                            start=True,
                            stop=True,
                            tile_position=(po, 0) if row_tiling else None,
                        )
                    dh.append(t)
                return dh

            def l1(dh, staged):
                """ACT stages PSUM halves; returns DVE L1 operands."""
                if staged:
                    sb0 = s0fpool.tile([128, 1024], F16)
                    nc.scalar.copy(sb0[:], dh[0][:])
                    sb1 = s1fpool.tile([128, 1024], F16)
                    nc.scalar.copy(sb1[:], dh[1][:])
                    return sb0, sb1
                sb1 = s1pool.tile([128, 1024], F32)
                nc.scalar.copy(sb1[:], dh[1][:])
                return dh[0], sb1

            def tail(t1, width, mdst):
                """Pairwise fp16 min chain over (width) groups of 16."""
                t1v = t1[:].rearrange("p (g n) -> p g n", n=16)
                t2 = t2pool.tile([128, width * 8], F16, tag="t2")
                t2v = t2[:].rearrange("p (g n) -> p g n", n=8)
                nc.vector.tensor_tensor(t2v, t1v[:, :, 0:8], t1v[:, :, 8:16], op=MIN)
                t3 = t3pool.tile([128, width * 4], F16, tag="t3")
                t3v = t3[:].rearrange("p (g n) -> p g n", n=4)
                nc.vector.tensor_tensor(t3v, t2v[:, :, 0:4], t2v[:, :, 4:8], op=MIN)
                t4 = t4pool.tile([128, width * 2], F16, tag="t4")
                t4v = t4[:].rearrange("p (g n) -> p g n", n=2)
                nc.vector.tensor_tensor(t4v, t3v[:, :, 0:2], t3v[:, :, 2:4], op=MIN)
                nc.vector.tensor_tensor(mdst, t4v[:, :, 0], t4v[:, :, 1], op=MIN)

            tile_counter = [0]

            def next_staged():
                idx = tile_counter[0]
                tile_counter[0] += 1
                return (idx * f16_frac) % 16 + f16_frac > 16

            def mean_chunk(b, side, MM, c):
                Z = zpool.tile([4, 512], F32, tag="z")
                nc.tensor.matmul(
                    Z[:], BO[:], MM[:, 512 * c : 512 * (c + 1)],
                    start=True, stop=True,
                )
                ZS = zspool.tile([4, 512], F32, tag="zs")
                nc.scalar.copy(ZS[:], Z[:])
                # SP queue: idle at the drain tail, unlike gpsimd which
                # also carries the input loads.
                nc.sync.dma_start(z_d.ap()[b, side, c], ZS[:])

            def run_side(b, side, L, R, last=False):
                MM = mpool.tile([128, 1024], F16, tag=f"m{side}")
                for ip in range(GBLK // 2):
                    if b == 0 and side == 0 and ip == 0:
                        # Startup: DVE's first L1 would wait on an ACT
                        # stage. Reduce the first pair straight from PSUM
                        # instead so DVE starts as soon as matmuls land.
                        for tau in range(2):
                            i = 2 * ip + tau
                            dh = d_tiles(b, i, L, R)
                            next_staged()
                            rs = []
                            for h in range(2):
                                r = s0fpool.tile([128, 64], F16, tag="r")
                                nc.vector.tensor_reduce(
                                    r[:].rearrange("p (g o) -> p g o", o=1),
                                    dh[h][:].rearrange("p (g n) -> p g n", n=16),
                                    axis=mybir.AxisListType.X,
                                    op=MIN,
                                )
                                rs.append(r)
                            nc.vector.tensor_tensor(
                                MM[:, 64 * i : 64 * (i + 1)],
                                rs[0][:], rs[1][:], op=MIN,
                            )
                        continue
                    t1 = t1pool.tile([128, 2048], F16, tag="t1")
                    for tau in range(2):
                        i = 2 * ip + tau
                        dh = d_tiles(b, i, L, R)
                        d0, sb1 = l1(dh, next_staged())
                        nc.vector.tensor_tensor(
                            t1[:, 1024 * tau : 1024 * (tau + 1)],
                            d0[:],
                            sb1[:],
                            op=MIN,
                        )
                    tail(t1, 128, MM[:, 128 * ip : 128 * (ip + 1)])
                    if last and ip == 5:
                        # Last side: its first mean half (pairs 0-3) is
                        # ready; emit it mid-side to shorten the drain tail.
                        mean_chunk(b, side, MM, 0)

                # Means: two wide matmuls, BO stationary, columns of MM
                # (tile, group) are contracted independently over the
                # 32-point partition blocks. Emitted one side late so the
                # PE stream never waits on the DVE min chain.
                def means():
                    for c in ([1] if last else [0, 1]):
                        mean_chunk(b, side, MM, c)

                return means

            pending = None
            for b in range(BPC):
                for side in range(2):
                    L, R = (XL, YR) if side == 0 else (YL, XR)
                    last = b == BPC - 1 and side == 1
                    means = run_side(b, side, L[:], R[:], last=last)
                    if pending is not None:
                        pending()
                    pending = means
            pending()

    nc.compile()
    return nc


def _host_prep(xyz1, xyz2):
    x = np.ascontiguousarray(xyz1, dtype=np.float32).reshape(B * G * N, 3)
    y = np.ascontiguousarray(xyz2, dtype=np.float32).reshape(B * G * N, 3)
    xa = np.empty((5, B * G * N), np.float16)
    xa[0] = (x * x).sum(-1)
    xa[1] = 1.0
    xa[2:5] = -2.0 * x.T
    ya = np.empty((5, B * G * N), np.float16)
    ya[0] = 1.0
    ya[1] = (y * y).sum(-1)
    ya[2:5] = y.T
    # rhs layout: (b, g, h nh) -> (b, h, g, nh)
    xar = (
        xa.reshape(5, B, G, 2, N // 2).transpose(0, 1, 3, 2, 4).reshape(5, -1)
    )
    yar = (
        ya.reshape(5, B, G, 2, N // 2).transpose(0, 1, 3, 2, 4).reshape(5, -1)
    )
    bo = np.zeros((128, 4), np.float16)
    for mblk in range(4):
        bo[32 * mblk : 32 * (mblk + 1), mblk] = 1.0 / 32
    return xa, ya, xar, yar, bo


def _assemble(z):
    """z [BPC, 2 sides, 2, 4, 512] -> out [BPC, 64, 64].

    Side 0: rows = g1 points -> z[b,0,c,sub,(i,g2)]: g1 = (c*8+i)*4+sub.
    Side 1: rows = g2 points -> z[b,1,c,sub,(j,g1)]: g2 = (c*8+j)*4+sub.
    """
    zc = z.reshape(BPC, 2, 2, 4, 8, 64)           # [b, side, c, sub, i, gcol]
    zc = zc.transpose(0, 1, 2, 4, 3, 5)           # [b, side, c, i, sub, gcol]
    zc = zc.reshape(BPC, 2, 64, 64)               # [b, side, grow, gcol]
    return zc[:, 0] + zc[:, 1].transpose(0, 2, 1)


def kernel(xyz1_matrix, xyz2_matrix):
    global LAST_EXEC_NS, LAST_RESULT
    xal, yal, xar, yar, bo = _host_prep(
        np.asarray(xyz1_matrix), np.asarray(xyz2_matrix)
    )
    nc = _build(_cfg_key(CONFIG))
    in_maps = []
    for c in range(NCORES):
        sl = slice(c * PTS, (c + 1) * PTS)
        in_maps.append(
            {
                "xal": np.ascontiguousarray(xal[:, sl]),
                "yal": np.ascontiguousarray(yal[:, sl]),
                "xar": np.ascontiguousarray(xar[:, sl]),
                "yar": np.ascontiguousarray(yar[:, sl]),
                "bo": bo,
            }
        )
    res = bass_utils.run_bass_kernel_spmd(
        nc, in_maps, core_ids=list(range(NCORES)), trace=TRACE, tmpdir=TRACE_DIR
    )
    LAST_RESULT = res
    LAST_EXEC_NS = res.exec_time_ns
    outs = [_assemble(r["z"]) for r in res.results]
    return np.concatenate(outs, axis=0).astype(np.float32)
